# revision 10
# baseline (speedup 1.0000x reference)
"""v2 kernel: bf16 datapath, dual-engine exp (Act native + DVE Schraudolph),
norm folding, deep pipelining.

Per-core work (query-sharded, 8 cores = 4 batches x 2 query-halves):
  x half  [256, 1152] -> q [128, 1152]
  y full  [512, 2304] -> k,v [128, 2304]
  attention 4 heads x (1152 q x 2304 k), out proj -> [256, 1152]

Norm folding:
  - x rms-norm applied to x before q-projection (bcx broadcast mul)
  - y rms-norm folded: K-norm into exp() per-partition scale;
    V-norm into the V^T PSUM->SBUF copy (Act Copy w/ per-partition scale)
  - q bias via Act Identity bias; k bias cancels in softmax; v bias folded
    into proj bias on host.
Exp split:
  - heads 0,1: Act engine native Exp (scale = SCALE*invy per partition)
  - heads 2,3: DVE Schraudolph bf16-bit exp: int16 = trunc(l*a_m + b),
    bitcast to bf16. a_m = SCALE*invy_m*128/ln2, b = 16256 - C.
"""

import os
import sys

import numpy as np

for _p in ("/root/.axon_site", "/root/.axon_site/_ro/trn_rl_repo",
           "/root/.axon_site/_ro/pypackages", "/opt/trn_rl_repo"):
    if _p not in sys.path and os.path.isdir(_p):
        sys.path.append(_p)

B = 4
CQ = 256
CKV = 512
N = 2304
NH = N // 2
DIM = 128
HEADS = 4
HD = 32
EPS = 1.5e-5
SCALE = HD ** -0.5
MT = N // 128
N_CORES = 8

# query windows: 4x256 + 1x128
W_OFF = [0, 256, 512, 768, 1024]
W_SZ = [256, 256, 256, 256, 128]
NWIN = len(W_OFF)

# Schraudolph bf16 exp constants (trunc-toward-zero calibration)
EXP_A = 128.0 / float(np.log(2.0))
EXP_B = 127.0 * 128.0 - 7.0

_EXEC = None


def _build_module(reps=1):
    from contextlib import ExitStack

    import concourse.tile as tile
    from concourse import bacc, mybir
    from concourse.masks import make_identity

    F32 = mybir.dt.float32
    BF16 = mybir.dt.bfloat16
    I16 = mybir.dt.int16
    AF = mybir.ActivationFunctionType

    nc = bacc.Bacc("TRN2", target_bir_lowering=False, debug=False,
                   num_devices=N_CORES)

    xb = nc.dram_tensor("xb", [2, 128, NH], BF16, kind="ExternalInput").ap()
    yb = nc.dram_tensor("yb", [4, 128, N], BF16, kind="ExternalInput").ap()
    qwT = nc.dram_tensor("qwT", [2, 128, 128], BF16, kind="ExternalInput").ap()
    kvwT = nc.dram_tensor("kvwT", [4, 128, 256], BF16, kind="ExternalInput").ap()
    pwT = nc.dram_tensor("pwT", [128, 256], BF16, kind="ExternalInput").ap()
    qb_d = nc.dram_tensor("qb", [128, 1], F32, kind="ExternalInput").ap()
    pb_d = nc.dram_tensor("pb", [128, 2], F32, kind="ExternalInput").ap()
    ones_d = nc.dram_tensor("ones", [128, 1], BF16, kind="ExternalInput").ap()
    invy_d = nc.dram_tensor("invy_rt", [1, N], F32, kind="ExternalOutput").ap()
    o_d = nc.dram_tensor("o", [2, 128, NH], F32, kind="ExternalOutput").ap()
    dbg = os.environ.get("KV2_DEBUG")
    if dbg:
        BF16_ = mybir.dt.bfloat16
        dbg_q = nc.dram_tensor("dbg_q", [128, NH], BF16_, kind="ExternalOutput").ap()
        dbg_k = nc.dram_tensor("dbg_k", [128, N], BF16_, kind="ExternalOutput").ap()
        dbg_invy = nc.dram_tensor("dbg_invy", [1, N], F32, kind="ExternalOutput").ap()
        dbg_invyT = nc.dram_tensor("dbg_invyT", [128, MT], F32, kind="ExternalOutput").ap()
        dbg_vT = nc.dram_tensor("dbg_vT", [128, MT, HEADS, 34], BF16_, kind="ExternalOutput").ap()
        dbg_attn = nc.dram_tensor("dbg_attn", [128, NH], BF16_, kind="ExternalOutput").ap()

    with tile.TileContext(nc) as tc, ExitStack() as ctx:
        consts = ctx.enter_context(tc.tile_pool(name="consts", bufs=1))

        ident = consts.tile([128, 128], BF16)
        qw_sb = consts.tile([128, 2, 128], BF16)
        kvw_sb = consts.tile([128, 4, 256], BF16)
        pw_sb = consts.tile([128, 256], BF16)
        qb_sb = consts.tile([128, 1], F32)
        pb_sb = consts.tile([128, 2], F32)
        ones_sb = consts.tile([128, 1], BF16)
        eps_sb = consts.tile([1, 1], F32)
        nc.sync.dma_start(qw_sb[:], qwT.rearrange("t p n -> p t n"))
        nc.sync.dma_start(kvw_sb[:], kvwT.rearrange("t p n -> p t n"))
        nc.sync.dma_start(qb_sb[:], qb_d[:])
        nc.gpsimd.dma_start(pw_sb[:], pwT[:])
        nc.gpsimd.dma_start(pb_sb[:], pb_d[:])
        nc.gpsimd.dma_start(ones_sb[:], ones_d[:])
        nc.gpsimd.memset(eps_sb[:], EPS)
        make_identity(nc, ident)

        for _rep in range(reps):
          with tc.tile_pool(name=f"persist{_rep}", bufs=1) as persist:
            q_sb = persist.tile([128, NH], BF16)
            k_sb = persist.tile([128, N], BF16)
            v_sb = persist.tile([128, N], BF16)
            vT_aug = persist.tile([128, MT, HEADS, 34], BF16)
            attn_sb = persist.tile([128, NH], BF16)
            o_sb = persist.tile([128, 2, NH], F32)
            invy = persist.tile([1, N], F32)
            invx_bf = persist.tile([1, NH], BF16)
            bcx = persist.tile([128, NH], BF16)
            invyT = persist.tile([128, MT], F32)
            aT_act = persist.tile([128, MT], F32)
            aT_dve = persist.tile([128, MT], F32)
            rms_y = persist.tile([1, N], F32)
            rms_x = persist.tile([1, NH], F32)

            with ExitStack() as s1:
                big = s1.enter_context(tc.tile_pool(name=f"big{_rep}", bufs=1))
                sqp = s1.enter_context(tc.tile_pool(name=f"sq{_rep}", bufs=2))
                ps_ss = s1.enter_context(
                    tc.tile_pool(name=f"ps_ss{_rep}", bufs=2, space="PSUM"))
                ps_q = s1.enter_context(
                    tc.tile_pool(name=f"ps_q{_rep}", bufs=2, space="PSUM"))
                ps_kv = s1.enter_context(
                    tc.tile_pool(name=f"ps_kv{_rep}", bufs=2, space="PSUM"))
                ps_t = s1.enter_context(
                    tc.tile_pool(name=f"ps_t{_rep}", bufs=2, space="PSUM"))

                x_t = [big.tile([128, NH], BF16, name=f"x{t}") for t in range(2)]
                y_t = [big.tile([128, N], BF16, name=f"y{t}") for t in range(4)]
                # All DMA triggers first: x (short q-critical-path), weights,
                # then y column-half-major.
                nc.sync.dma_start(x_t[0][:], xb[0])
                nc.gpsimd.dma_start(x_t[1][:], xb[1])
                for jh in range(2):
                    for t in range(4):
                        sl = slice(jh * NH, (jh + 1) * NH)
                        eng = nc.sync if t % 2 == 0 else nc.gpsimd
                        eng.dma_start(y_t[t][:, sl], yb[t][:, sl])
                # gpsimd: ones-fill vT_aug (augmented ones row at col 32)
                nc.gpsimd.memset(vT_aug[:], 1.0)

                # x squares -> ssq_x -> invx(bf16) -> bcx   (q norm factors)
                x2 = big.tile([128, NH], BF16, name="x2")
                sq0 = sqp.tile([128, NH], BF16, name="sq0", tag="sq0")
                sq1 = sqp.tile([128, NH], BF16, name="sq1", tag="sq1")
                nc.vector.tensor_mul(sq0[:], x_t[0][:], x_t[0][:])
                nc.vector.tensor_mul(sq1[:], x_t[1][:], x_t[1][:])
                nc.vector.tensor_add(x2[:], sq0[:], sq1[:])
                x_chunks = [(0, 512), (512, 512), (1024, 128)]
                for c0, cw in x_chunks:
                    ps = ps_ss.tile([1, 512], F32, name="ssps", tag="ssps")
                    nc.tensor.matmul(out=ps[0:1, 0:cw], lhsT=ones_sb[:],
                                     rhs=x2[:, c0:c0 + cw],
                                     start=True, stop=True)
                    nc.scalar.activation(out=rms_x[0:1, c0:c0 + cw],
                                         in_=ps[0:1, 0:cw], func=AF.Sqrt,
                                         scale=1.0 / CQ, bias=eps_sb[:])
                    with nc.allow_low_precision(reason="x inv-rms in bf16"):
                        nc.vector.reciprocal(invx_bf[0:1, c0:c0 + cw],
                                             rms_x[0:1, c0:c0 + cw])
                nc.gpsimd.partition_broadcast(bcx[:], invx_bf[:], channels=128)

                # k projection (QK critical path; PE order: before ssq_y)
                y_chunks = [(0, 512), (512, 512), (1024, 512), (1536, 512),
                            (2048, 256)]
                for c0, cw in y_chunks:
                    ps = ps_kv.tile([128, 512], F32, name="kvps", tag="kvps")
                    for t in range(4):
                        nc.tensor.matmul(
                            out=ps[:, 0:cw], lhsT=kvw_sb[:, t, 0:128],
                            rhs=y_t[t][:, c0:c0 + cw],
                            start=(t == 0), stop=(t == 3))
                    nc.scalar.activation(out=k_sb[:, c0:c0 + cw],
                                         in_=ps[:, 0:cw], func=AF.Copy)

                # q projection on RAW x (starts right after x DMA), then
                # per-column x-norm scale (DVE) and bias add (Act Identity)
                qt_sb = big.tile([128, NH], BF16, name="qt")
                for c0, cw in x_chunks:
                    ps = ps_q.tile([128, 512], F32, name="qps", tag="qps")
                    for t in range(2):
                        nc.tensor.matmul(out=ps[:, 0:cw],
                                         lhsT=qw_sb[:, t, :],
                                         rhs=x_t[t][:, c0:c0 + cw],
                                         start=(t == 0), stop=(t == 1))
                    nc.vector.tensor_mul(qt_sb[:, c0:c0 + cw], ps[:, 0:cw],
                                         bcx[:, c0:c0 + cw])
                    nc.scalar.activation(out=q_sb[:, c0:c0 + cw],
                                         in_=qt_sb[:, c0:c0 + cw],
                                         func=AF.Identity,
                                         bias=qb_sb[:], scale=1.0)

                # y squares -> ssq_y -> invy -> invyT roundtrip -> aT
                y2 = [big.tile([128, N], BF16, name=f"y2_{p}") for p in range(2)]
                for jh in range(2):
                    for p in range(2):
                        sl = slice(jh * NH, (jh + 1) * NH)
                        sq0 = sqp.tile([128, NH], BF16, name="sq0", tag="sq0")
                        sq1 = sqp.tile([128, NH], BF16, name="sq1", tag="sq1")
                        nc.vector.tensor_mul(sq0[:], y_t[2 * p][:, sl],
                                             y_t[2 * p][:, sl])
                        nc.vector.tensor_mul(sq1[:], y_t[2 * p + 1][:, sl],
                                             y_t[2 * p + 1][:, sl])
                        nc.vector.tensor_add(y2[p][:, sl], sq0[:], sq1[:])
                for c0, cw in y_chunks:
                    ps = ps_ss.tile([1, 512], F32, name="ssps", tag="ssps")
                    for p in range(2):
                        nc.tensor.matmul(out=ps[0:1, 0:cw], lhsT=ones_sb[:],
                                         rhs=y2[p][:, c0:c0 + cw],
                                         start=(p == 0), stop=(p == 1))
                    nc.scalar.activation(out=rms_y[0:1, c0:c0 + cw],
                                         in_=ps[0:1, 0:cw], func=AF.Sqrt,
                                         scale=1.0 / CKV, bias=eps_sb[:])
                    nc.vector.reciprocal(invy[0:1, c0:c0 + cw],
                                         rms_y[0:1, c0:c0 + cw])
                # invy [1, N] -> invyT [128, MT] via DRAM roundtrip
                # (2-D scratch: 1-D DRAM tensors fail the NEFF loader, and
                # SBUF->SBUF partition-scatter DMA corrupts data on HW)
                nc.sync.dma_start(invy_d[:], invy[:])
                nc.sync.dma_start(invyT[:],
                                  invy_d.rearrange("o (t p) -> p (o t)", p=128))
                nc.vector.tensor_scalar_mul(aT_act[:], invyT[:], SCALE)
                nc.vector.tensor_scalar_mul(aT_dve[:], invyT[:],
                                            SCALE * EXP_A)

                # v projection + transposes (only needed by PV, 1 mt behind)
                for c0, cw in y_chunks:
                    ps = ps_kv.tile([128, 512], F32, name="kvps", tag="kvps")
                    for t in range(4):
                        nc.tensor.matmul(
                            out=ps[:, 0:cw], lhsT=kvw_sb[:, t, 128:256],
                            rhs=y_t[t][:, c0:c0 + cw],
                            start=(t == 0), stop=(t == 3))
                    nc.scalar.activation(out=v_sb[:, c0:c0 + cw],
                                         in_=ps[:, 0:cw], func=AF.Copy)
                    for mt in range(c0 // 128, (c0 + cw) // 128):
                        # bank-padded tile: avoid two transposes sharing
                        # one PSUM zero region
                        ps2 = ps_t.tile([128, 1024], BF16, name="tps",
                                        tag="tps")[:, 0:128]
                        nc.tensor.transpose(
                            ps2[:], v_sb[:, mt * 128:(mt + 1) * 128],
                            ident[:])
                        # V-norm folded into the copy (scale by invy_m)
                        nc.scalar.activation(
                            out=vT_aug[:, mt, :, 0:32],
                            in_=ps2.rearrange("p (h d) -> p h d", h=4),
                            func=AF.Copy,
                            scale=invyT[:, mt:mt + 1])

            if os.environ.get("KV2_STAGE") == "prologue":
                nc.vector.memset(o_sb[:], 0.0)
                nc.sync.dma_start(o_d[0], o_sb[:, 0, :])
                nc.sync.dma_start(o_d[1], o_sb[:, 1, :])
                continue

            # ---- attention windows ----
            # PSUM: one shared ring (tag "qd", bufs=3) of [128, 2, 512] tiles
            # (2 banks each; one bank per head -> every matmul group starts
            # its own zero region, which HW requires for row-band
            # tile_position groups) serves qdA/qdB/pj = 6 banks; pvq ring
            # bufs=2 x 1 bank = 2 banks. Total 8.
            with ExitStack() as s2:
                psw = s2.enter_context(
                    tc.tile_pool(name=f"psw{_rep}", bufs=3, space="PSUM"))
                pvp = s2.enter_context(
                    tc.tile_pool(name=f"pv{_rep}", bufs=2, space="PSUM"))
                pTAp = s2.enter_context(tc.tile_pool(name=f"pTA{_rep}", bufs=3))
                pTBp = s2.enter_context(tc.tile_pool(name=f"pTB{_rep}", bufs=3))
                smp = s2.enter_context(tc.tile_pool(name=f"sm{_rep}", bufs=4))

                # pvq packing: head -> (row base, col base)
                HPOS = [(0, 0), (64, 0), (0, 256), (64, 256)]

                def qk(w, mt, qd, hh):
                    # hh: (h0, h1) head pair into qd [128, 2, 512]; each head
                    # gets its own PSUM bank and its own start=True group
                    # (HW rejects start=False at a different row-band
                    # tile_position than the group's start).
                    nsl = slice(W_OFF[w], W_OFF[w] + W_SZ[w])
                    for i, h in enumerate(hh):
                        nc.tensor.matmul(
                            out=qd[:, i, 0:W_SZ[w]],
                            lhsT=k_sb[32 * h:32 * h + 32,
                                      mt * 128:(mt + 1) * 128],
                            rhs=q_sb[32 * h:32 * h + 32, nsl],
                            start=True, stop=True,
                            tile_position=(32 * h, 0))

                def pv(w, mt, pvq, pTA, pTB):
                    # Shared PSUM bank, 4 groups at (row 0|64) x (col 0|256).
                    # At mt==0 only h0 (partitions 0:33) and h1 (64:97) start
                    # their partition-range zero regions; h2/h3 land on
                    # pending-zero cells with start=False.
                    cw = W_SZ[w]
                    for h in range(HEADS):
                        rb, cb = HPOS[h]
                        src = pTA if h < 2 else pTB
                        rhs = src[:, h % 2, 0:cw]
                        nc.tensor.matmul(
                            out=pvq[rb:rb + 33, cb:cb + cw],
                            lhsT=vT_aug[:, mt, h, 0:33],
                            rhs=rhs,
                            start=(mt == 0 and h < 2), stop=(mt == MT - 1),
                            tile_position=(0, rb),
                            skip_group_check=True)

                def divide_steps(w, pvq):
                    # softmax divide spread over mt slots: merged per-row-band
                    # reciprocal (h-even at row 32, h-odd at row 96), then
                    # per-head multiplies.
                    cw = W_SZ[w]
                    nsl = slice(W_OFF[w], W_OFF[w] + cw)
                    rdbs = {}

                    def band(rb):
                        def f():
                            rd = smp.tile([1, 512], F32, name="rd", tag="rd")
                            rdb = smp.tile([32, 512], F32, name="rdb",
                                           tag="rdb")
                            if cw == 256:
                                src = pvq[rb + 32:rb + 33, :]
                                nc.vector.reciprocal(rd[:], src)
                                nc.gpsimd.partition_broadcast(
                                    rdb[:], rd[:], channels=32)
                            else:
                                src = pvq.rearrange("p (c n) -> p c n", c=2)[
                                    rb + 32:rb + 33, :, 0:cw]
                                dst = rd.rearrange("p (c n) -> p c n", c=2)[
                                    :, :, 0:cw]
                                nc.vector.reciprocal(dst, src)
                                nc.gpsimd.partition_broadcast(
                                    rdb.rearrange("p (c n) -> p c n", c=2)[
                                        :, :, 0:cw],
                                    dst, channels=32)
                            rdbs[rb] = rdb
                        return f

                    def mul(h):
                        def f():
                            rb, cb = HPOS[h]
                            nc.vector.tensor_mul(
                                attn_sb[32 * h:32 * h + 32, nsl],
                                pvq[rb:rb + 32, cb:cb + cw],
                                rdbs[rb][:, cb:cb + cw])
                        return f

                    return [band(0), band(64), mul(0), mul(1), mul(2), mul(3)]

                def proj(w):
                    cw = W_SZ[w]
                    nsl = slice(W_OFF[w], W_OFF[w] + cw)
                    ps = psw.tile([128, 2, 512], F32, name="pj", tag="qd")
                    for ct in range(2):
                        nc.tensor.matmul(
                            out=ps[:, ct, 0:cw],
                            lhsT=pw_sb[:, ct * 128:(ct + 1) * 128],
                            rhs=attn_sb[:, nsl], start=True, stop=True)
                        nc.scalar.activation(out=o_sb[:, ct, nsl],
                                             in_=ps[:, ct, 0:cw],
                                             func=AF.Identity,
                                             bias=pb_sb[:, ct:ct + 1],
                                             scale=1.0)
                        nc.gpsimd.dma_start(o_d[ct][:, nsl], o_sb[:, ct, nsl])

                prev = None  # (w, mt, pvq, pTA, pTB) pending PV
                pvq_of = {}
                pending = []  # trailing divide/proj steps of previous window
                nwin_lim = int(os.environ.get("KV2_STAGE_NWIN", NWIN))
                for w in range(nwin_lim):
                    cw = W_SZ[w]
                    pvq = pvp.tile([128, 512], F32, name="pvq", tag="pvq")
                    pvq_of[w] = pvq
                    for mt in range(MT):
                        qdA = psw.tile([128, 2, 512], F32, name="qdA",
                                       tag="qd")
                        qdB = psw.tile([128, 2, 512], F32, name="qdB",
                                       tag="qd")
                        qk(w, mt, qdA, (0, 1))
                        qk(w, mt, qdB, (2, 3))
                        pTA = pTAp.tile([128, 2, 256], BF16, name="pTA",
                                        tag="pTA")
                        nc.scalar.activation(out=pTA[:, :, 0:cw],
                                             in_=qdA[:, :, 0:cw],
                                             func=AF.Exp,
                                             scale=aT_act[:, mt:mt + 1])
                        pTB = pTBp.tile([128, 2, 256], I16, name="pTB",
                                        tag="pTB")
                        if os.environ.get("KV2_ACT_ONLY"):
                            nc.scalar.activation(
                                out=pTB.bitcast(BF16)[:, :, 0:cw],
                                in_=qdB[:, :, 0:cw], func=AF.Exp,
                                scale=aT_act[:, mt:mt + 1])
                        else:
                            nc.vector.tensor_scalar(
                                out=pTB[:, :, 0:cw], in0=qdB[:, :, 0:cw],
                                scalar1=aT_dve[:, mt:mt + 1], scalar2=EXP_B,
                                op0=mybir.AluOpType.mult,
                                op1=mybir.AluOpType.add)
                        if prev is not None:
                            pv(*prev)
                        prev = (w, mt, pvq, pTA, pTB.bitcast(BF16))
                        # trailing work of previous window, spread over slots
                        if 1 <= mt <= len(pending):
                            pending[mt - 1]()
                    if w > 0:
                        pending = []
                    pending += divide_steps(w, pvq)
                    pending.append(lambda w=w: proj(w))
                pv(*prev)
                for step in pending:
                    step()
                if dbg:
                    nc.sync.dma_start(dbg_q[:], q_sb[:])
                    nc.sync.dma_start(dbg_k[:], k_sb[:])
                    nc.sync.dma_start(dbg_invy[:], invy[:])
                    nc.sync.dma_start(dbg_invyT[:], invyT[:])
                    nc.sync.dma_start(dbg_vT[:], vT_aug[:])
                    nc.sync.dma_start(dbg_attn[:], attn_sb[:])

    nc.compile()
    return nc


def _make_in_maps(inputs):
    import ml_dtypes
    bf16 = ml_dtypes.bfloat16

    x = np.ascontiguousarray(np.asarray(inputs["x"], np.float32)).reshape(B, CQ, N)
    y = np.ascontiguousarray(np.asarray(inputs["y"], np.float32)).reshape(B, CKV, N)
    q_w = np.asarray(inputs["q_w"], np.float32)
    kv_w = np.asarray(inputs["kv_w"], np.float32)
    qn = np.asarray(inputs["q_norm_w"], np.float32)
    kvn = np.asarray(inputs["kv_norm_w"], np.float32)
    q_b = np.asarray(inputs["q_b"], np.float32)
    kv_b = np.asarray(inputs["kv_b"], np.float32)
    proj_w = np.asarray(inputs["proj_w"], np.float32)
    proj_b = np.asarray(inputs["proj_b"], np.float32)

    qwT = np.ascontiguousarray((q_w * qn[None, :]).T).reshape(2, 128, 128)
    kvwT = np.ascontiguousarray((kv_w * kvn[None, :]).T).reshape(4, 128, 256)
    pwT = np.ascontiguousarray(proj_w.T)
    pb_eff = (proj_b + proj_w @ kv_b[128:]).astype(np.float32)
    shared = {
        "qwT": qwT.astype(bf16), "kvwT": kvwT.astype(bf16),
        "pwT": pwT.astype(bf16),
        "qb": q_b.reshape(128, 1),
        "pb": np.ascontiguousarray(pb_eff.reshape(2, 128).T),
        "ones": np.ones((128, 1), bf16),
    }
    in_maps = []
    for c in range(N_CORES):
        b, half = c // 2, c % 2
        xbv = np.ascontiguousarray(
            x[b][:, half * NH:(half + 1) * NH]).reshape(2, 128, NH)
        ybv = np.ascontiguousarray(y[b]).reshape(4, 128, N)
        in_maps.append({"xb": xbv.astype(bf16), "yb": ybv.astype(bf16),
                        **shared})
    return in_maps


class _Executor:
    """Compile once; run the SPMD kernel via PJRT/axon on 8 cores."""

    def __init__(self, reps=1):
        import jax
        from jax.sharding import Mesh, PartitionSpec
        from jax.experimental.shard_map import shard_map

        from concourse import bass2jax, mybir

        self.nc = _build_module(reps=reps)
        bass2jax.install_neuronx_cc_hook()

        partition_name = (self.nc.partition_id_tensor.name
                          if self.nc.partition_id_tensor else None)
        in_names, out_names, out_avals, zero_outs = [], [], [], []
        for alloc in self.nc.m.functions[0].allocations:
            if not isinstance(alloc, mybir.MemoryLocationSet):
                continue
            name = alloc.memorylocations[0].name
            if alloc.kind == "ExternalInput":
                if name != partition_name:
                    in_names.append(name)
            elif alloc.kind == "ExternalOutput":
                out_names.append(name)
                shape = tuple(alloc.tensor_shape)
                dtype = mybir.dt.np(alloc.dtype)
                out_avals.append(jax.core.ShapedArray(shape, dtype))
                zero_outs.append(np.zeros(shape, dtype))
        self.in_names, self.out_names = in_names, out_names
        self.out_avals, self.zero_outs = out_avals, zero_outs
        n_params, n_outs = len(in_names), len(out_names)
        all_names = list(in_names + out_names)
        if partition_name is not None:
            all_names.append(partition_name)
        all_names = tuple(all_names)

        def make_body(reps):
            def _body(*args):
                operands = list(args)
                if partition_name is not None:
                    operands.append(bass2jax.partition_id_tensor())
                return tuple(bass2jax._bass_exec_p.bind(
                    *operands,
                    out_avals=tuple(out_avals),
                    in_names=all_names,
                    out_names=tuple(out_names),
                    lowering_input_output_aliases=(),
                    sim_require_finite=False,
                    sim_require_nnan=False,
                    nc=self.nc,
                ))
            return _body

        devices = jax.devices()[:N_CORES]
        self.mesh = Mesh(np.asarray(devices), ("core",))
        in_specs = (PartitionSpec("core"),) * (n_params + n_outs)
        out_specs = (PartitionSpec("core"),) * n_outs
        self._jits = {}

        def get_jit(reps):
            if reps not in self._jits:
                self._jits[reps] = jax.jit(
                    shard_map(make_body(reps), mesh=self.mesh,
                              in_specs=in_specs, out_specs=out_specs,
                              check_rep=False),
                    keep_unused=True)
            return self._jits[reps]

        self._get_jit = get_jit

    def _concat_args(self, in_maps):
        cat = [np.concatenate([np.asarray(m[n]) for m in in_maps], axis=0)
               for n in self.in_names]
        cat += [np.concatenate([z] * len(in_maps), axis=0)
                for z in self.zero_outs]
        return cat

    def run(self, in_maps, reps=1):
        args = self._concat_args(in_maps)
        outs = self._get_jit(reps)(*args)
        n = len(in_maps)
        return [
            {name: np.asarray(outs[i]).reshape(n, *self.out_avals[i].shape)[c]
             for i, name in enumerate(self.out_names)}
            for c in range(n)
        ]


def _get_executor():
    global _EXEC
    if _EXEC is None:
        _EXEC = _Executor()
    return _EXEC


def kernel(**inputs):
    ex = _get_executor()
    res = ex.run(_make_in_maps(inputs))
    out = np.empty((B, CQ, N), np.float32)
    for c in range(N_CORES):
        b, half = c // 2, c % 2
        out[b][:, half * NH:(half + 1) * NH] = res[c]["o"].reshape(CQ, NH)
    return out.reshape(B, CQ, 48, 48)


# revision 11
# speedup vs baseline: 1.0077x; 1.0077x over previous
"""v2 kernel: bf16 datapath, dual-engine exp (Act native + DVE Schraudolph),
norm folding, deep pipelining.

Per-core work (query-sharded, 8 cores = 4 batches x 2 query-halves):
  x half  [256, 1152] -> q [128, 1152]
  y full  [512, 2304] -> k,v [128, 2304]
  attention 4 heads x (1152 q x 2304 k), out proj -> [256, 1152]

Norm folding:
  - x rms-norm applied to x before q-projection (bcx broadcast mul)
  - y rms-norm folded: K-norm into exp() per-partition scale;
    V-norm into the V^T PSUM->SBUF copy (Act Copy w/ per-partition scale)
  - q bias via Act Identity bias; k bias cancels in softmax; v bias folded
    into proj bias on host.
Exp split:
  - heads 0,1: Act engine native Exp (scale = SCALE*invy per partition)
  - heads 2,3: DVE Schraudolph bf16-bit exp: int16 = trunc(l*a_m + b),
    bitcast to bf16. a_m = SCALE*invy_m*128/ln2, b = 16256 - C.
"""

import os
import sys

import numpy as np

for _p in ("/root/.axon_site", "/root/.axon_site/_ro/trn_rl_repo",
           "/root/.axon_site/_ro/pypackages", "/opt/trn_rl_repo"):
    if _p not in sys.path and os.path.isdir(_p):
        sys.path.append(_p)

B = 4
CQ = 256
CKV = 512
N = 2304
NH = N // 2
DIM = 128
HEADS = 4
HD = 32
EPS = 1.5e-5
SCALE = HD ** -0.5
MT = N // 128
N_CORES = 8

# query windows: 4x256 + 1x128
W_OFF = [0, 256, 512, 768, 1024]
W_SZ = [256, 256, 256, 256, 128]
NWIN = len(W_OFF)

# Schraudolph bf16 exp constants (trunc-toward-zero calibration)
EXP_A = 128.0 / float(np.log(2.0))
EXP_B = 127.0 * 128.0 - 7.0

_EXEC = None


def _build_module(reps=1):
    from contextlib import ExitStack

    import concourse.tile as tile
    from concourse import bacc, mybir
    from concourse.masks import make_identity

    F32 = mybir.dt.float32
    BF16 = mybir.dt.bfloat16
    I16 = mybir.dt.int16
    AF = mybir.ActivationFunctionType

    nc = bacc.Bacc("TRN2", target_bir_lowering=False, debug=False,
                   num_devices=N_CORES)

    xb = nc.dram_tensor("xb", [2, 128, NH], BF16, kind="ExternalInput").ap()
    yb = nc.dram_tensor("yb", [4, 128, N], BF16, kind="ExternalInput").ap()
    qwT = nc.dram_tensor("qwT", [2, 128, 128], BF16, kind="ExternalInput").ap()
    kvwT = nc.dram_tensor("kvwT", [4, 128, 256], BF16, kind="ExternalInput").ap()
    pwT = nc.dram_tensor("pwT", [128, 256], BF16, kind="ExternalInput").ap()
    qb_d = nc.dram_tensor("qb", [128, 1], F32, kind="ExternalInput").ap()
    pb_d = nc.dram_tensor("pb", [128, 2], F32, kind="ExternalInput").ap()
    ones_d = nc.dram_tensor("ones", [128, 1], BF16, kind="ExternalInput").ap()
    invy_d = nc.dram_tensor("invy_rt", [1, N], F32, kind="ExternalOutput").ap()
    o_d = nc.dram_tensor("o", [2, 128, NH], F32, kind="ExternalOutput").ap()
    dbg = os.environ.get("KV2_DEBUG")
    if dbg:
        BF16_ = mybir.dt.bfloat16
        dbg_q = nc.dram_tensor("dbg_q", [128, NH], BF16_, kind="ExternalOutput").ap()
        dbg_k = nc.dram_tensor("dbg_k", [128, N], BF16_, kind="ExternalOutput").ap()
        dbg_invy = nc.dram_tensor("dbg_invy", [1, N], F32, kind="ExternalOutput").ap()
        dbg_invyT = nc.dram_tensor("dbg_invyT", [128, MT], F32, kind="ExternalOutput").ap()
        dbg_vT = nc.dram_tensor("dbg_vT", [128, MT, HEADS, 34], BF16_, kind="ExternalOutput").ap()
        dbg_attn = nc.dram_tensor("dbg_attn", [128, NH], BF16_, kind="ExternalOutput").ap()

    with tile.TileContext(nc) as tc, ExitStack() as ctx:
        consts = ctx.enter_context(tc.tile_pool(name="consts", bufs=1))

        ident = consts.tile([128, 128], BF16)
        qw_sb = consts.tile([128, 2, 128], BF16)
        kvw_sb = consts.tile([128, 4, 256], BF16)
        pw_sb = consts.tile([128, 256], BF16)
        qb_sb = consts.tile([128, 1], F32)
        pb_sb = consts.tile([128, 2], F32)
        ones_sb = consts.tile([128, 1], BF16)
        eps_sb = consts.tile([1, 1], F32)
        nc.sync.dma_start(qw_sb[:], qwT.rearrange("t p n -> p t n"))
        nc.sync.dma_start(kvw_sb[:], kvwT.rearrange("t p n -> p t n"))
        nc.sync.dma_start(qb_sb[:], qb_d[:])
        nc.gpsimd.dma_start(pw_sb[:], pwT[:])
        nc.gpsimd.dma_start(pb_sb[:], pb_d[:])
        nc.gpsimd.dma_start(ones_sb[:], ones_d[:])
        nc.gpsimd.memset(eps_sb[:], EPS)
        make_identity(nc, ident)

        for _rep in range(reps):
          with tc.tile_pool(name=f"persist{_rep}", bufs=1) as persist:
            q_sb = persist.tile([128, NH], BF16)
            k_sb = persist.tile([128, N], BF16)
            v_sb = persist.tile([128, N], BF16)
            vT_aug = persist.tile([128, MT, HEADS, 34], BF16)
            attn_sb = persist.tile([128, NH], BF16)
            o_sb = persist.tile([128, 2, NH], F32)
            invy = persist.tile([1, N], F32)
            invx_bf = persist.tile([1, NH], BF16)
            bcx = persist.tile([128, NH], BF16)
            invyT = persist.tile([128, MT], F32)
            aT_act = persist.tile([128, MT], F32)
            aT_dve = persist.tile([128, MT], F32)
            rms_y = persist.tile([1, N], F32)
            rms_x = persist.tile([1, NH], F32)

            with ExitStack() as s1:
                big = s1.enter_context(tc.tile_pool(name=f"big{_rep}", bufs=1))
                sqp = s1.enter_context(tc.tile_pool(name=f"sq{_rep}", bufs=2))
                ps_ss = s1.enter_context(
                    tc.tile_pool(name=f"ps_ss{_rep}", bufs=2, space="PSUM"))
                ps_q = s1.enter_context(
                    tc.tile_pool(name=f"ps_q{_rep}", bufs=2, space="PSUM"))
                ps_kv = s1.enter_context(
                    tc.tile_pool(name=f"ps_kv{_rep}", bufs=2, space="PSUM"))
                ps_t = s1.enter_context(
                    tc.tile_pool(name=f"ps_t{_rep}", bufs=2, space="PSUM"))

                x_t = [big.tile([128, NH], BF16, name=f"x{t}") for t in range(2)]
                y_t = [big.tile([128, N], BF16, name=f"y{t}") for t in range(4)]
                # All DMA triggers first: x (short q-critical-path), weights,
                # then y column-half-major.
                nc.sync.dma_start(x_t[0][:], xb[0])
                nc.gpsimd.dma_start(x_t[1][:], xb[1])
                y_q = [nc.sync, nc.gpsimd, nc.scalar, nc.sync]
                for jh in range(2):
                    for t in range(4):
                        sl = slice(jh * NH, (jh + 1) * NH)
                        y_q[t].dma_start(y_t[t][:, sl], yb[t][:, sl])
                # gpsimd: ones-fill vT_aug (augmented ones row at col 32)
                nc.gpsimd.memset(vT_aug[:], 1.0)

                # x squares -> ssq_x -> invx(bf16) -> bcx   (q norm factors)
                x2 = big.tile([128, NH], BF16, name="x2")
                sq0 = sqp.tile([128, NH], BF16, name="sq0", tag="sq0")
                sq1 = sqp.tile([128, NH], BF16, name="sq1", tag="sq1")
                nc.vector.tensor_mul(sq0[:], x_t[0][:], x_t[0][:])
                nc.vector.tensor_mul(sq1[:], x_t[1][:], x_t[1][:])
                nc.vector.tensor_add(x2[:], sq0[:], sq1[:])
                x_chunks = [(0, 512), (512, 512), (1024, 128)]
                for c0, cw in x_chunks:
                    ps = ps_ss.tile([1, 512], F32, name="ssps", tag="ssps")
                    nc.tensor.matmul(out=ps[0:1, 0:cw], lhsT=ones_sb[:],
                                     rhs=x2[:, c0:c0 + cw],
                                     start=True, stop=True)
                    nc.scalar.activation(out=rms_x[0:1, c0:c0 + cw],
                                         in_=ps[0:1, 0:cw], func=AF.Sqrt,
                                         scale=1.0 / CQ, bias=eps_sb[:])
                    with nc.allow_low_precision(reason="x inv-rms in bf16"):
                        nc.vector.reciprocal(invx_bf[0:1, c0:c0 + cw],
                                             rms_x[0:1, c0:c0 + cw])
                nc.gpsimd.partition_broadcast(bcx[:], invx_bf[:], channels=128)

                # k projection (QK critical path; PE order: before ssq_y)
                y_chunks = [(0, 512), (512, 512), (1024, 512), (1536, 512),
                            (2048, 256)]
                for c0, cw in y_chunks:
                    ps = ps_kv.tile([128, 512], F32, name="kvps", tag="kvps")
                    for t in range(4):
                        nc.tensor.matmul(
                            out=ps[:, 0:cw], lhsT=kvw_sb[:, t, 0:128],
                            rhs=y_t[t][:, c0:c0 + cw],
                            start=(t == 0), stop=(t == 3))
                    nc.scalar.activation(out=k_sb[:, c0:c0 + cw],
                                         in_=ps[:, 0:cw], func=AF.Copy)

                # q projection on RAW x (starts right after x DMA), then
                # per-column x-norm scale (DVE) and bias add (Act Identity)
                qt_sb = big.tile([128, NH], BF16, name="qt")
                for c0, cw in x_chunks:
                    ps = ps_q.tile([128, 512], F32, name="qps", tag="qps")
                    for t in range(2):
                        nc.tensor.matmul(out=ps[:, 0:cw],
                                         lhsT=qw_sb[:, t, :],
                                         rhs=x_t[t][:, c0:c0 + cw],
                                         start=(t == 0), stop=(t == 1))
                    nc.vector.tensor_mul(qt_sb[:, c0:c0 + cw], ps[:, 0:cw],
                                         bcx[:, c0:c0 + cw])
                    nc.scalar.activation(out=q_sb[:, c0:c0 + cw],
                                         in_=qt_sb[:, c0:c0 + cw],
                                         func=AF.Identity,
                                         bias=qb_sb[:], scale=1.0)

                # y squares -> ssq_y -> invy -> invyT roundtrip -> aT
                y2 = [big.tile([128, N], BF16, name=f"y2_{p}") for p in range(2)]
                for jh in range(2):
                    for p in range(2):
                        sl = slice(jh * NH, (jh + 1) * NH)
                        sq0 = sqp.tile([128, NH], BF16, name="sq0", tag="sq0")
                        sq1 = sqp.tile([128, NH], BF16, name="sq1", tag="sq1")
                        nc.vector.tensor_mul(sq0[:], y_t[2 * p][:, sl],
                                             y_t[2 * p][:, sl])
                        nc.vector.tensor_mul(sq1[:], y_t[2 * p + 1][:, sl],
                                             y_t[2 * p + 1][:, sl])
                        nc.vector.tensor_add(y2[p][:, sl], sq0[:], sq1[:])
                for c0, cw in y_chunks:
                    ps = ps_ss.tile([1, 512], F32, name="ssps", tag="ssps")
                    for p in range(2):
                        nc.tensor.matmul(out=ps[0:1, 0:cw], lhsT=ones_sb[:],
                                         rhs=y2[p][:, c0:c0 + cw],
                                         start=(p == 0), stop=(p == 1))
                    nc.scalar.activation(out=rms_y[0:1, c0:c0 + cw],
                                         in_=ps[0:1, 0:cw], func=AF.Sqrt,
                                         scale=1.0 / CKV, bias=eps_sb[:])
                    nc.vector.reciprocal(invy[0:1, c0:c0 + cw],
                                         rms_y[0:1, c0:c0 + cw])
                # invy [1, N] -> invyT [128, MT] via DRAM roundtrip
                # (2-D scratch: 1-D DRAM tensors fail the NEFF loader, and
                # SBUF->SBUF partition-scatter DMA corrupts data on HW)
                nc.sync.dma_start(invy_d[:], invy[:])
                nc.sync.dma_start(invyT[:],
                                  invy_d.rearrange("o (t p) -> p (o t)", p=128))
                nc.vector.tensor_scalar_mul(aT_act[:], invyT[:], SCALE)
                nc.vector.tensor_scalar_mul(aT_dve[:], invyT[:],
                                            SCALE * EXP_A)

                # v projection + transposes (only needed by PV, 1 mt behind)
                for c0, cw in y_chunks:
                    ps = ps_kv.tile([128, 512], F32, name="kvps", tag="kvps")
                    for t in range(4):
                        nc.tensor.matmul(
                            out=ps[:, 0:cw], lhsT=kvw_sb[:, t, 128:256],
                            rhs=y_t[t][:, c0:c0 + cw],
                            start=(t == 0), stop=(t == 3))
                    nc.scalar.activation(out=v_sb[:, c0:c0 + cw],
                                         in_=ps[:, 0:cw], func=AF.Copy)
                    for mt in range(c0 // 128, (c0 + cw) // 128):
                        # bank-padded tile: avoid two transposes sharing
                        # one PSUM zero region
                        ps2 = ps_t.tile([128, 1024], BF16, name="tps",
                                        tag="tps")[:, 0:128]
                        nc.tensor.transpose(
                            ps2[:], v_sb[:, mt * 128:(mt + 1) * 128],
                            ident[:])
                        # V-norm folded into the copy (scale by invy_m)
                        nc.scalar.activation(
                            out=vT_aug[:, mt, :, 0:32],
                            in_=ps2.rearrange("p (h d) -> p h d", h=4),
                            func=AF.Copy,
                            scale=invyT[:, mt:mt + 1])

            if os.environ.get("KV2_STAGE") == "prologue":
                nc.vector.memset(o_sb[:], 0.0)
                nc.sync.dma_start(o_d[0], o_sb[:, 0, :])
                nc.sync.dma_start(o_d[1], o_sb[:, 1, :])
                continue

            # ---- attention windows ----
            # PSUM: one shared ring (tag "qd", bufs=3) of [128, 2, 512] tiles
            # (2 banks each; one bank per head -> every matmul group starts
            # its own zero region, which HW requires for row-band
            # tile_position groups) serves qdA/qdB/pj = 6 banks; pvq ring
            # bufs=2 x 1 bank = 2 banks. Total 8.
            with ExitStack() as s2:
                psw = s2.enter_context(
                    tc.tile_pool(name=f"psw{_rep}", bufs=3, space="PSUM"))
                pvp = s2.enter_context(
                    tc.tile_pool(name=f"pv{_rep}", bufs=2, space="PSUM"))
                pTAp = s2.enter_context(tc.tile_pool(name=f"pTA{_rep}", bufs=3))
                pTBp = s2.enter_context(tc.tile_pool(name=f"pTB{_rep}", bufs=3))
                smp = s2.enter_context(tc.tile_pool(name=f"sm{_rep}", bufs=4))

                # pvq packing: head -> (row base, col base)
                HPOS = [(0, 0), (64, 0), (0, 256), (64, 256)]

                def qk(w, mt, qd, hh):
                    # hh: (h0, h1) head pair into qd [128, 2, 512]; each head
                    # gets its own PSUM bank and its own start=True group
                    # (HW rejects start=False at a different row-band
                    # tile_position than the group's start).
                    nsl = slice(W_OFF[w], W_OFF[w] + W_SZ[w])
                    for i, h in enumerate(hh):
                        nc.tensor.matmul(
                            out=qd[:, i, 0:W_SZ[w]],
                            lhsT=k_sb[32 * h:32 * h + 32,
                                      mt * 128:(mt + 1) * 128],
                            rhs=q_sb[32 * h:32 * h + 32, nsl],
                            start=True, stop=True,
                            tile_position=(32 * h, 0))

                def pv(w, mt, pvq, pTA, pTB):
                    # Shared PSUM bank, 4 groups at (row 0|64) x (col 0|256).
                    # At mt==0 only h0 (partitions 0:33) and h1 (64:97) start
                    # their partition-range zero regions; h2/h3 land on
                    # pending-zero cells with start=False.
                    cw = W_SZ[w]
                    for h in range(HEADS):
                        rb, cb = HPOS[h]
                        src = pTA if h < 2 else pTB
                        rhs = src[:, h % 2, 0:cw]
                        nc.tensor.matmul(
                            out=pvq[rb:rb + 33, cb:cb + cw],
                            lhsT=vT_aug[:, mt, h, 0:33],
                            rhs=rhs,
                            start=(mt == 0 and h < 2), stop=(mt == MT - 1),
                            tile_position=(0, rb),
                            skip_group_check=True)

                def divide_steps(w, pvq):
                    # softmax divide spread over mt slots: merged per-row-band
                    # reciprocal (h-even at row 32, h-odd at row 96), then
                    # per-head multiplies.
                    cw = W_SZ[w]
                    nsl = slice(W_OFF[w], W_OFF[w] + cw)
                    rdbs = {}

                    def band(rb):
                        def f():
                            rd = smp.tile([1, 512], F32, name="rd", tag="rd")
                            rdb = smp.tile([32, 512], F32, name="rdb",
                                           tag="rdb")
                            if cw == 256:
                                src = pvq[rb + 32:rb + 33, :]
                                nc.vector.reciprocal(rd[:], src)
                                nc.gpsimd.partition_broadcast(
                                    rdb[:], rd[:], channels=32)
                            else:
                                src = pvq.rearrange("p (c n) -> p c n", c=2)[
                                    rb + 32:rb + 33, :, 0:cw]
                                dst = rd.rearrange("p (c n) -> p c n", c=2)[
                                    :, :, 0:cw]
                                nc.vector.reciprocal(dst, src)
                                nc.gpsimd.partition_broadcast(
                                    rdb.rearrange("p (c n) -> p c n", c=2)[
                                        :, :, 0:cw],
                                    dst, channels=32)
                            rdbs[rb] = rdb
                        return f

                    def mul(h):
                        def f():
                            rb, cb = HPOS[h]
                            nc.vector.tensor_mul(
                                attn_sb[32 * h:32 * h + 32, nsl],
                                pvq[rb:rb + 32, cb:cb + cw],
                                rdbs[rb][:, cb:cb + cw])
                        return f

                    return [band(0), band(64), mul(0), mul(1), mul(2), mul(3)]

                def proj(w):
                    cw = W_SZ[w]
                    nsl = slice(W_OFF[w], W_OFF[w] + cw)
                    ps = psw.tile([128, 2, 512], F32, name="pj", tag="qd")
                    for ct in range(2):
                        nc.tensor.matmul(
                            out=ps[:, ct, 0:cw],
                            lhsT=pw_sb[:, ct * 128:(ct + 1) * 128],
                            rhs=attn_sb[:, nsl], start=True, stop=True)
                        nc.scalar.activation(out=o_sb[:, ct, nsl],
                                             in_=ps[:, ct, 0:cw],
                                             func=AF.Identity,
                                             bias=pb_sb[:, ct:ct + 1],
                                             scale=1.0)
                        nc.gpsimd.dma_start(o_d[ct][:, nsl], o_sb[:, ct, nsl])

                prev = None  # (w, mt, pvq, pTA, pTB) pending PV
                pvq_of = {}
                pending = []  # trailing divide/proj steps of previous window
                nwin_lim = int(os.environ.get("KV2_STAGE_NWIN", NWIN))
                for w in range(nwin_lim):
                    cw = W_SZ[w]
                    pvq = pvp.tile([128, 512], F32, name="pvq", tag="pvq")
                    pvq_of[w] = pvq
                    for mt in range(MT):
                        qdA = psw.tile([128, 2, 512], F32, name="qdA",
                                       tag="qd")
                        qdB = psw.tile([128, 2, 512], F32, name="qdB",
                                       tag="qd")
                        qk(w, mt, qdA, (0, 1))
                        qk(w, mt, qdB, (2, 3))
                        pTA = pTAp.tile([128, 2, 256], BF16, name="pTA",
                                        tag="pTA")
                        nc.scalar.activation(out=pTA[:, :, 0:cw],
                                             in_=qdA[:, :, 0:cw],
                                             func=AF.Exp,
                                             scale=aT_act[:, mt:mt + 1])
                        pTB = pTBp.tile([128, 2, 256], I16, name="pTB",
                                        tag="pTB")
                        if os.environ.get("KV2_ACT_ONLY"):
                            nc.scalar.activation(
                                out=pTB.bitcast(BF16)[:, :, 0:cw],
                                in_=qdB[:, :, 0:cw], func=AF.Exp,
                                scale=aT_act[:, mt:mt + 1])
                        else:
                            nc.vector.tensor_scalar(
                                out=pTB[:, :, 0:cw], in0=qdB[:, :, 0:cw],
                                scalar1=aT_dve[:, mt:mt + 1], scalar2=EXP_B,
                                op0=mybir.AluOpType.mult,
                                op1=mybir.AluOpType.add)
                        if prev is not None:
                            pv(*prev)
                        prev = (w, mt, pvq, pTA, pTB.bitcast(BF16))
                        # trailing work of previous window, spread over slots
                        if 1 <= mt <= len(pending):
                            pending[mt - 1]()
                    if w > 0:
                        pending = []
                    pending += divide_steps(w, pvq)
                    pending.append(lambda w=w: proj(w))
                pv(*prev)
                for step in pending:
                    step()
                if dbg:
                    nc.sync.dma_start(dbg_q[:], q_sb[:])
                    nc.sync.dma_start(dbg_k[:], k_sb[:])
                    nc.sync.dma_start(dbg_invy[:], invy[:])
                    nc.sync.dma_start(dbg_invyT[:], invyT[:])
                    nc.sync.dma_start(dbg_vT[:], vT_aug[:])
                    nc.sync.dma_start(dbg_attn[:], attn_sb[:])

    nc.compile()
    return nc


def _make_in_maps(inputs):
    import ml_dtypes
    bf16 = ml_dtypes.bfloat16

    x = np.ascontiguousarray(np.asarray(inputs["x"], np.float32)).reshape(B, CQ, N)
    y = np.ascontiguousarray(np.asarray(inputs["y"], np.float32)).reshape(B, CKV, N)
    q_w = np.asarray(inputs["q_w"], np.float32)
    kv_w = np.asarray(inputs["kv_w"], np.float32)
    qn = np.asarray(inputs["q_norm_w"], np.float32)
    kvn = np.asarray(inputs["kv_norm_w"], np.float32)
    q_b = np.asarray(inputs["q_b"], np.float32)
    kv_b = np.asarray(inputs["kv_b"], np.float32)
    proj_w = np.asarray(inputs["proj_w"], np.float32)
    proj_b = np.asarray(inputs["proj_b"], np.float32)

    qwT = np.ascontiguousarray((q_w * qn[None, :]).T).reshape(2, 128, 128)
    kvwT = np.ascontiguousarray((kv_w * kvn[None, :]).T).reshape(4, 128, 256)
    pwT = np.ascontiguousarray(proj_w.T)
    pb_eff = (proj_b + proj_w @ kv_b[128:]).astype(np.float32)
    shared = {
        "qwT": qwT.astype(bf16), "kvwT": kvwT.astype(bf16),
        "pwT": pwT.astype(bf16),
        "qb": q_b.reshape(128, 1),
        "pb": np.ascontiguousarray(pb_eff.reshape(2, 128).T),
        "ones": np.ones((128, 1), bf16),
    }
    in_maps = []
    for c in range(N_CORES):
        b, half = c // 2, c % 2
        xbv = np.ascontiguousarray(
            x[b][:, half * NH:(half + 1) * NH]).reshape(2, 128, NH)
        ybv = np.ascontiguousarray(y[b]).reshape(4, 128, N)
        in_maps.append({"xb": xbv.astype(bf16), "yb": ybv.astype(bf16),
                        **shared})
    return in_maps


class _Executor:
    """Compile once; run the SPMD kernel via PJRT/axon on 8 cores."""

    def __init__(self, reps=1):
        import jax
        from jax.sharding import Mesh, PartitionSpec
        from jax.experimental.shard_map import shard_map

        from concourse import bass2jax, mybir

        self.nc = _build_module(reps=reps)
        bass2jax.install_neuronx_cc_hook()

        partition_name = (self.nc.partition_id_tensor.name
                          if self.nc.partition_id_tensor else None)
        in_names, out_names, out_avals, zero_outs = [], [], [], []
        for alloc in self.nc.m.functions[0].allocations:
            if not isinstance(alloc, mybir.MemoryLocationSet):
                continue
            name = alloc.memorylocations[0].name
            if alloc.kind == "ExternalInput":
                if name != partition_name:
                    in_names.append(name)
            elif alloc.kind == "ExternalOutput":
                out_names.append(name)
                shape = tuple(alloc.tensor_shape)
                dtype = mybir.dt.np(alloc.dtype)
                out_avals.append(jax.core.ShapedArray(shape, dtype))
                zero_outs.append(np.zeros(shape, dtype))
        self.in_names, self.out_names = in_names, out_names
        self.out_avals, self.zero_outs = out_avals, zero_outs
        n_params, n_outs = len(in_names), len(out_names)
        all_names = list(in_names + out_names)
        if partition_name is not None:
            all_names.append(partition_name)
        all_names = tuple(all_names)

        def make_body(reps):
            def _body(*args):
                operands = list(args)
                if partition_name is not None:
                    operands.append(bass2jax.partition_id_tensor())
                return tuple(bass2jax._bass_exec_p.bind(
                    *operands,
                    out_avals=tuple(out_avals),
                    in_names=all_names,
                    out_names=tuple(out_names),
                    lowering_input_output_aliases=(),
                    sim_require_finite=False,
                    sim_require_nnan=False,
                    nc=self.nc,
                ))
            return _body

        devices = jax.devices()[:N_CORES]
        self.mesh = Mesh(np.asarray(devices), ("core",))
        in_specs = (PartitionSpec("core"),) * (n_params + n_outs)
        out_specs = (PartitionSpec("core"),) * n_outs
        self._jits = {}

        def get_jit(reps):
            if reps not in self._jits:
                self._jits[reps] = jax.jit(
                    shard_map(make_body(reps), mesh=self.mesh,
                              in_specs=in_specs, out_specs=out_specs,
                              check_rep=False),
                    keep_unused=True)
            return self._jits[reps]

        self._get_jit = get_jit

    def _concat_args(self, in_maps):
        cat = [np.concatenate([np.asarray(m[n]) for m in in_maps], axis=0)
               for n in self.in_names]
        cat += [np.concatenate([z] * len(in_maps), axis=0)
                for z in self.zero_outs]
        return cat

    def run(self, in_maps, reps=1):
        args = self._concat_args(in_maps)
        outs = self._get_jit(reps)(*args)
        n = len(in_maps)
        return [
            {name: np.asarray(outs[i]).reshape(n, *self.out_avals[i].shape)[c]
             for i, name in enumerate(self.out_names)}
            for c in range(n)
        ]


def _get_executor():
    global _EXEC
    if _EXEC is None:
        _EXEC = _Executor()
    return _EXEC


def kernel(**inputs):
    ex = _get_executor()
    res = ex.run(_make_in_maps(inputs))
    out = np.empty((B, CQ, N), np.float32)
    for c in range(N_CORES):
        b, half = c // 2, c % 2
        out[b][:, half * NH:(half + 1) * NH] = res[c]["o"].reshape(CQ, NH)
    return out.reshape(B, CQ, 48, 48)


# revision 15
# speedup vs baseline: 1.0996x; 1.0912x over previous
"""v2 kernel: bf16 datapath, dual-engine exp (Act native + DVE Schraudolph),
norm folding, deep pipelining.

Per-core work (query-sharded, 8 cores = 4 batches x 2 query-halves):
  x half  [256, 1152] -> q [128, 1152]
  y full  [512, 2304] -> k,v [128, 2304]
  attention 4 heads x (1152 q x 2304 k), out proj -> [256, 1152]

Norm folding:
  - x rms-norm applied to x before q-projection (bcx broadcast mul)
  - y rms-norm folded: K-norm into exp() per-partition scale;
    V-norm into the V^T PSUM->SBUF copy (Act Copy w/ per-partition scale)
  - q bias via Act Identity bias; k bias cancels in softmax; v bias folded
    into proj bias on host.
Exp split:
  - heads 0,1: Act engine native Exp (scale = SCALE*invy per partition)
  - heads 2,3: DVE Schraudolph bf16-bit exp: int16 = trunc(l*a_m + b),
    bitcast to bf16. a_m = SCALE*invy_m*128/ln2, b = 16256 - C.
"""

import os
import sys

import numpy as np

for _p in ("/root/.axon_site", "/root/.axon_site/_ro/trn_rl_repo",
           "/root/.axon_site/_ro/pypackages", "/opt/trn_rl_repo"):
    if _p not in sys.path and os.path.isdir(_p):
        sys.path.append(_p)

B = 4
CQ = 256
CKV = 512
N = 2304
NH = N // 2
DIM = 128
HEADS = 4
HD = 32
EPS = 1.5e-5
SCALE = HD ** -0.5
MT = N // 128
N_CORES = 8

# query windows: 3x384
W_OFF = [0, 384, 768]
W_SZ = [384, 384, 384]
NWIN = len(W_OFF)

# Schraudolph bf16 exp constants (trunc-toward-zero calibration)
EXP_A = 128.0 / float(np.log(2.0))
EXP_B = 127.0 * 128.0 - 7.0

_EXEC = None


def _build_module(reps=1):
    from contextlib import ExitStack

    import concourse.tile as tile
    from concourse import bacc, mybir
    from concourse.masks import make_identity

    F32 = mybir.dt.float32
    BF16 = mybir.dt.bfloat16
    I16 = mybir.dt.int16
    AF = mybir.ActivationFunctionType

    nc = bacc.Bacc("TRN2", target_bir_lowering=False, debug=False,
                   num_devices=N_CORES)

    xb = nc.dram_tensor("xb", [2, 128, NH], BF16, kind="ExternalInput").ap()
    yb = nc.dram_tensor("yb", [4, 128, N], BF16, kind="ExternalInput").ap()
    qwT = nc.dram_tensor("qwT", [2, 128, 128], BF16, kind="ExternalInput").ap()
    kvwT = nc.dram_tensor("kvwT", [4, 128, 256], BF16, kind="ExternalInput").ap()
    pwT = nc.dram_tensor("pwT", [128, 256], BF16, kind="ExternalInput").ap()
    qb_d = nc.dram_tensor("qb", [128, 1], F32, kind="ExternalInput").ap()
    pb_d = nc.dram_tensor("pb", [128, 2], F32, kind="ExternalInput").ap()
    ones_d = nc.dram_tensor("ones", [128, 1], BF16, kind="ExternalInput").ap()
    invy_d = nc.dram_tensor("invy_rt", [1, N], F32, kind="ExternalOutput").ap()
    o_d = nc.dram_tensor("o", [2, 128, NH], F32, kind="ExternalOutput").ap()
    dbg = os.environ.get("KV2_DEBUG")
    if dbg:
        BF16_ = mybir.dt.bfloat16
        dbg_q = nc.dram_tensor("dbg_q", [128, NH], BF16_, kind="ExternalOutput").ap()
        dbg_k = nc.dram_tensor("dbg_k", [128, N], BF16_, kind="ExternalOutput").ap()
        dbg_invy = nc.dram_tensor("dbg_invy", [1, N], F32, kind="ExternalOutput").ap()
        dbg_invyT = nc.dram_tensor("dbg_invyT", [128, MT], F32, kind="ExternalOutput").ap()
        dbg_vT = nc.dram_tensor("dbg_vT", [128, MT, HEADS, 34], BF16_, kind="ExternalOutput").ap()
        dbg_attn = nc.dram_tensor("dbg_attn", [128, NH], BF16_, kind="ExternalOutput").ap()

    with tile.TileContext(nc) as tc, ExitStack() as ctx:
        consts = ctx.enter_context(tc.tile_pool(name="consts", bufs=1))

        ident = consts.tile([128, 128], BF16)
        qw_sb = consts.tile([128, 2, 128], BF16)
        kvw_sb = consts.tile([128, 4, 256], BF16)
        pw_sb = consts.tile([128, 256], BF16)
        qb_sb = consts.tile([128, 1], F32)
        pb_sb = consts.tile([128, 2], F32)
        ones_sb = consts.tile([128, 1], BF16)
        eps_sb = consts.tile([1, 1], F32)
        nc.sync.dma_start(qw_sb[:], qwT.rearrange("t p n -> p t n"))
        nc.sync.dma_start(kvw_sb[:], kvwT.rearrange("t p n -> p t n"))
        nc.sync.dma_start(qb_sb[:], qb_d[:])
        nc.gpsimd.dma_start(pw_sb[:], pwT[:])
        nc.gpsimd.dma_start(pb_sb[:], pb_d[:])
        nc.gpsimd.dma_start(ones_sb[:], ones_d[:])
        nc.gpsimd.memset(eps_sb[:], EPS)
        make_identity(nc, ident)

        for _rep in range(reps):
          with tc.tile_pool(name=f"persist{_rep}", bufs=1) as persist:
            q_sb = persist.tile([128, NH], BF16)
            k_sb = persist.tile([128, N], BF16)
            v_sb = persist.tile([128, N], BF16)
            vT_aug = persist.tile([128, MT, HEADS, 34], BF16)
            attn_sb = persist.tile([128, NH], BF16)
            o_sb = persist.tile([128, 2, NH], F32)
            invy = persist.tile([1, N], F32)
            invx_bf = persist.tile([1, NH], BF16)
            bcx = persist.tile([128, NH], BF16)
            invyT = persist.tile([128, MT], F32)
            aT_act = persist.tile([128, MT], F32)
            aT_dve = persist.tile([128, MT], F32)
            rms_y = persist.tile([1, N], F32)
            rms_x = persist.tile([1, NH], F32)

            with ExitStack() as s1:
                big = s1.enter_context(tc.tile_pool(name=f"big{_rep}", bufs=1))
                sqp = s1.enter_context(tc.tile_pool(name=f"sq{_rep}", bufs=2))
                ps_ss = s1.enter_context(
                    tc.tile_pool(name=f"ps_ss{_rep}", bufs=2, space="PSUM"))
                ps_q = s1.enter_context(
                    tc.tile_pool(name=f"ps_q{_rep}", bufs=2, space="PSUM"))
                ps_kv = s1.enter_context(
                    tc.tile_pool(name=f"ps_kv{_rep}", bufs=2, space="PSUM"))
                ps_t = s1.enter_context(
                    tc.tile_pool(name=f"ps_t{_rep}", bufs=2, space="PSUM"))

                x_t = [big.tile([128, NH], BF16, name=f"x{t}") for t in range(2)]
                y_t = [big.tile([128, N], BF16, name=f"y{t}") for t in range(4)]
                # All DMA triggers first: x (short q-critical-path), weights,
                # then y column-half-major.
                nc.sync.dma_start(x_t[0][:], xb[0])
                nc.gpsimd.dma_start(x_t[1][:], xb[1])
                y_q = [nc.sync, nc.gpsimd, nc.scalar, nc.sync]
                for jh in range(2):
                    for t in range(4):
                        sl = slice(jh * NH, (jh + 1) * NH)
                        y_q[t].dma_start(y_t[t][:, sl], yb[t][:, sl])
                # gpsimd: ones-fill vT_aug (augmented ones row at col 32)
                nc.gpsimd.memset(vT_aug[:], 1.0)

                # x squares -> ssq_x -> invx(bf16) -> bcx   (q norm factors)
                x2 = big.tile([128, NH], BF16, name="x2")
                sq0 = sqp.tile([128, NH], BF16, name="sq0", tag="sq0")
                sq1 = sqp.tile([128, NH], BF16, name="sq1", tag="sq1")
                nc.vector.tensor_mul(sq0[:], x_t[0][:], x_t[0][:])
                nc.vector.tensor_mul(sq1[:], x_t[1][:], x_t[1][:])
                nc.vector.tensor_add(x2[:], sq0[:], sq1[:])
                x_chunks = [(0, 512), (512, 512), (1024, 128)]
                for c0, cw in x_chunks:
                    ps = ps_ss.tile([1, 512], F32, name="ssps", tag="ssps")
                    nc.tensor.matmul(out=ps[0:1, 0:cw], lhsT=ones_sb[:],
                                     rhs=x2[:, c0:c0 + cw],
                                     start=True, stop=True)
                    nc.scalar.activation(out=rms_x[0:1, c0:c0 + cw],
                                         in_=ps[0:1, 0:cw], func=AF.Sqrt,
                                         scale=1.0 / CQ, bias=eps_sb[:])
                    with nc.allow_low_precision(reason="x inv-rms in bf16"):
                        nc.vector.reciprocal(invx_bf[0:1, c0:c0 + cw],
                                             rms_x[0:1, c0:c0 + cw])
                nc.gpsimd.partition_broadcast(bcx[:], invx_bf[:], channels=128)

                # k projection (QK critical path; PE order: before ssq_y)
                y_chunks = [(0, 512), (512, 512), (1024, 512), (1536, 512),
                            (2048, 256)]
                for c0, cw in y_chunks:
                    ps = ps_kv.tile([128, 512], F32, name="kvps", tag="kvps")
                    for t in range(4):
                        nc.tensor.matmul(
                            out=ps[:, 0:cw], lhsT=kvw_sb[:, t, 0:128],
                            rhs=y_t[t][:, c0:c0 + cw],
                            start=(t == 0), stop=(t == 3))
                    nc.scalar.activation(out=k_sb[:, c0:c0 + cw],
                                         in_=ps[:, 0:cw], func=AF.Copy)

                # q projection on RAW x (starts right after x DMA), then
                # per-column x-norm scale (DVE) and bias add (Act Identity)
                qt_sb = big.tile([128, NH], BF16, name="qt")
                for c0, cw in x_chunks:
                    ps = ps_q.tile([128, 512], F32, name="qps", tag="qps")
                    for t in range(2):
                        nc.tensor.matmul(out=ps[:, 0:cw],
                                         lhsT=qw_sb[:, t, :],
                                         rhs=x_t[t][:, c0:c0 + cw],
                                         start=(t == 0), stop=(t == 1))
                    nc.vector.tensor_mul(qt_sb[:, c0:c0 + cw], ps[:, 0:cw],
                                         bcx[:, c0:c0 + cw])
                    nc.scalar.activation(out=q_sb[:, c0:c0 + cw],
                                         in_=qt_sb[:, c0:c0 + cw],
                                         func=AF.Identity,
                                         bias=qb_sb[:], scale=1.0)

                # y squares -> ssq_y -> invy -> invyT roundtrip -> aT
                y2 = [big.tile([128, N], BF16, name=f"y2_{p}") for p in range(2)]
                for jh in range(2):
                    for p in range(2):
                        sl = slice(jh * NH, (jh + 1) * NH)
                        sq0 = sqp.tile([128, NH], BF16, name="sq0", tag="sq0")
                        sq1 = sqp.tile([128, NH], BF16, name="sq1", tag="sq1")
                        nc.vector.tensor_mul(sq0[:], y_t[2 * p][:, sl],
                                             y_t[2 * p][:, sl])
                        nc.vector.tensor_mul(sq1[:], y_t[2 * p + 1][:, sl],
                                             y_t[2 * p + 1][:, sl])
                        nc.vector.tensor_add(y2[p][:, sl], sq0[:], sq1[:])
                for c0, cw in y_chunks:
                    ps = ps_ss.tile([1, 512], F32, name="ssps", tag="ssps")
                    for p in range(2):
                        nc.tensor.matmul(out=ps[0:1, 0:cw], lhsT=ones_sb[:],
                                         rhs=y2[p][:, c0:c0 + cw],
                                         start=(p == 0), stop=(p == 1))
                    nc.scalar.activation(out=rms_y[0:1, c0:c0 + cw],
                                         in_=ps[0:1, 0:cw], func=AF.Sqrt,
                                         scale=1.0 / CKV, bias=eps_sb[:])
                    nc.vector.reciprocal(invy[0:1, c0:c0 + cw],
                                         rms_y[0:1, c0:c0 + cw])
                # invy [1, N] -> invyT [128, MT] via DRAM roundtrip
                # (2-D scratch: 1-D DRAM tensors fail the NEFF loader, and
                # SBUF->SBUF partition-scatter DMA corrupts data on HW)
                nc.sync.dma_start(invy_d[:], invy[:])
                nc.sync.dma_start(invyT[:],
                                  invy_d.rearrange("o (t p) -> p (o t)", p=128))
                nc.vector.tensor_scalar_mul(aT_act[:], invyT[:], SCALE)
                nc.vector.tensor_scalar_mul(aT_dve[:], invyT[:],
                                            SCALE * EXP_A)

                # v projection + transposes (only needed by PV, 1 mt behind)
                for c0, cw in y_chunks:
                    ps = ps_kv.tile([128, 512], F32, name="kvps", tag="kvps")
                    for t in range(4):
                        nc.tensor.matmul(
                            out=ps[:, 0:cw], lhsT=kvw_sb[:, t, 128:256],
                            rhs=y_t[t][:, c0:c0 + cw],
                            start=(t == 0), stop=(t == 3))
                    nc.scalar.activation(out=v_sb[:, c0:c0 + cw],
                                         in_=ps[:, 0:cw], func=AF.Copy)
                    for mt in range(c0 // 128, (c0 + cw) // 128):
                        # bank-padded tile: avoid two transposes sharing
                        # one PSUM zero region
                        ps2 = ps_t.tile([128, 1024], BF16, name="tps",
                                        tag="tps")[:, 0:128]
                        nc.tensor.transpose(
                            ps2[:], v_sb[:, mt * 128:(mt + 1) * 128],
                            ident[:])
                        # V-norm folded into the copy (scale by invy_m)
                        nc.scalar.activation(
                            out=vT_aug[:, mt, :, 0:32],
                            in_=ps2.rearrange("p (h d) -> p h d", h=4),
                            func=AF.Copy,
                            scale=invyT[:, mt:mt + 1])

            if os.environ.get("KV2_STAGE") == "prologue":
                nc.vector.memset(o_sb[:], 0.0)
                nc.sync.dma_start(o_d[0], o_sb[:, 0, :])
                nc.sync.dma_start(o_d[1], o_sb[:, 1, :])
                continue

            # ---- attention windows ----
            # PSUM: one shared ring (tag "qd", bufs=3) of [128, 2, 512] tiles
            # (2 banks each; one bank per head -> every matmul group starts
            # its own zero region, which HW requires for row-band
            # tile_position groups) serves qdA/qdB/pj = 6 banks; pvq ring
            # bufs=2 x 1 bank = 2 banks. Total 8.
            with ExitStack() as s2:
                psw = s2.enter_context(
                    tc.tile_pool(name=f"psw{_rep}", bufs=3, space="PSUM"))
                pvp = s2.enter_context(
                    tc.tile_pool(name=f"pv{_rep}", bufs=1, space="PSUM"))
                pTAp = s2.enter_context(tc.tile_pool(name=f"pTA{_rep}", bufs=3))
                pTBp = s2.enter_context(tc.tile_pool(name=f"pTB{_rep}", bufs=3))
                smp = s2.enter_context(tc.tile_pool(name=f"sm{_rep}", bufs=4))

                # pvq packing: head -> (row base, bank)
                HPOS = [(0, 0), (64, 0), (0, 1), (64, 1)]
                # SBUF staging for the softmax divide (Act copies right after
                # the window's last PV free the single PSUM accumulator for
                # the next window). Both row bands land at base partition 0
                # so every divide operand shares base 0 (partition_broadcast
                # to offset bases is broken on HW, and the verifier requires
                # equal bases for two-SBUF-input TensorTensor).
                pvs = [persist.tile([33, 2, 512], F32, name=f"pvs{i}")
                       for i in range(2)]

                def qk(w, mt, qd, hh):
                    # hh: (h0, h1) head pair into qd [128, 2, 512]; each head
                    # gets its own PSUM bank and its own start=True group
                    # (HW rejects start=False at a different row-band
                    # tile_position than the group's start).
                    nsl = slice(W_OFF[w], W_OFF[w] + W_SZ[w])
                    for i, h in enumerate(hh):
                        nc.tensor.matmul(
                            out=qd[:, i, 0:W_SZ[w]],
                            lhsT=k_sb[32 * h:32 * h + 32,
                                      mt * 128:(mt + 1) * 128],
                            rhs=q_sb[32 * h:32 * h + 32, nsl],
                            start=True, stop=True,
                            tile_position=(32 * h, 0))

                def pv(w, mt, pvq, pTA, pTB):
                    # pvq [128, 2, 512]: 4 groups at (row 0|64) x (bank 0|1),
                    # each with its own zero region -> all start at mt==0.
                    cw = W_SZ[w]
                    for h in range(HEADS):
                        rb, bk = HPOS[h]
                        src = pTA if h < 2 else pTB
                        rhs = src[:, h % 2, 0:cw]
                        nc.tensor.matmul(
                            out=pvq[rb:rb + 33, bk, 0:cw],
                            lhsT=vT_aug[:, mt, h, 0:33],
                            rhs=rhs,
                            start=(mt == 0), stop=(mt == MT - 1),
                            tile_position=(0, rb),
                            skip_group_check=True)

                def divide_steps(w, pvq):
                    # Stage (Act) both pvq row bands down to base-0 SBUF,
                    # freeing the PSUM accumulator early; then reciprocals,
                    # base-0 broadcasts, and per-head multiplies from SBUF.
                    cw = W_SZ[w]
                    nsl = slice(W_OFF[w], W_OFF[w] + cw)
                    rdbs = {}

                    def stage(i):
                        def f():
                            rb = 64 * i
                            nc.scalar.activation(
                                out=pvs[i][:, :, 0:cw],
                                in_=pvq[rb:rb + 33, :, 0:cw],
                                func=AF.Copy)
                        return f

                    def band(i):
                        def f():
                            rd = smp.tile([1, 2, 512], F32, name="rd",
                                          tag="rd")
                            rdb = smp.tile([32, 2, 512], F32, name="rdb",
                                           tag="rdb")
                            nc.vector.reciprocal(
                                rd[0:1, :, 0:cw],
                                pvs[i][32:33, :, 0:cw])
                            nc.gpsimd.partition_broadcast(
                                rdb[:, :, 0:cw], rd[0:1, :, 0:cw],
                                channels=32)
                            rdbs[i] = rdb
                        return f

                    def mul(h):
                        def f():
                            i, bk = h % 2, h // 2
                            nc.vector.tensor_mul(
                                attn_sb[32 * h:32 * h + 32, nsl],
                                pvs[i][0:32, bk, 0:cw],
                                rdbs[i][:, bk, 0:cw])
                        return f

                    return [stage(0), stage(1), band(0), band(1),
                            mul(0), mul(1), mul(2), mul(3)]

                def proj(w):
                    cw = W_SZ[w]
                    nsl = slice(W_OFF[w], W_OFF[w] + cw)
                    ps = psw.tile([128, 2, 512], F32, name="pj", tag="qd")
                    for ct in range(2):
                        nc.tensor.matmul(
                            out=ps[:, ct, 0:cw],
                            lhsT=pw_sb[:, ct * 128:(ct + 1) * 128],
                            rhs=attn_sb[:, nsl], start=True, stop=True)
                        nc.scalar.activation(out=o_sb[:, ct, nsl],
                                             in_=ps[:, ct, 0:cw],
                                             func=AF.Identity,
                                             bias=pb_sb[:, ct:ct + 1],
                                             scale=1.0)
                        nc.gpsimd.dma_start(o_d[ct][:, nsl], o_sb[:, ct, nsl])

                prev = None  # (w, mt, pvq, pTA, pTB) pending PV
                pvq_of = {}
                pending = []  # trailing divide/proj steps of previous window
                nwin_lim = int(os.environ.get("KV2_STAGE_NWIN", NWIN))
                for w in range(nwin_lim):
                    cw = W_SZ[w]
                    pvq = pvp.tile([128, 2, 512], F32, name="pvq",
                                   tag="pvq")
                    pvq_of[w] = pvq
                    for mt in range(MT):
                        qdA = psw.tile([128, 2, 512], F32, name="qdA",
                                       tag="qd")
                        qdB = psw.tile([128, 2, 512], F32, name="qdB",
                                       tag="qd")
                        qk(w, mt, qdA, (0, 1))
                        qk(w, mt, qdB, (2, 3))
                        pTA = pTAp.tile([128, 2, 384], BF16, name="pTA",
                                        tag="pTA")
                        nc.scalar.activation(out=pTA[:, :, 0:cw],
                                             in_=qdA[:, :, 0:cw],
                                             func=AF.Exp,
                                             scale=aT_act[:, mt:mt + 1])
                        pTB = pTBp.tile([128, 2, 384], I16, name="pTB",
                                        tag="pTB")
                        if os.environ.get("KV2_ACT_ONLY"):
                            nc.scalar.activation(
                                out=pTB.bitcast(BF16)[:, :, 0:cw],
                                in_=qdB[:, :, 0:cw], func=AF.Exp,
                                scale=aT_act[:, mt:mt + 1])
                        else:
                            nc.vector.tensor_scalar(
                                out=pTB[:, :, 0:cw], in0=qdB[:, :, 0:cw],
                                scalar1=aT_dve[:, mt:mt + 1], scalar2=EXP_B,
                                op0=mybir.AluOpType.mult,
                                op1=mybir.AluOpType.add)
                        if prev is not None:
                            pv(*prev)
                        prev = (w, mt, pvq, pTA, pTB.bitcast(BF16))
                        # trailing work of previous window, spread over slots
                        if 1 <= mt <= len(pending):
                            pending[mt - 1]()
                    if w > 0:
                        pending = []
                    pending += divide_steps(w, pvq)
                    pending.append(lambda w=w: proj(w))
                pv(*prev)
                for step in pending:
                    step()
                if dbg:
                    nc.sync.dma_start(dbg_q[:], q_sb[:])
                    nc.sync.dma_start(dbg_k[:], k_sb[:])
                    nc.sync.dma_start(dbg_invy[:], invy[:])
                    nc.sync.dma_start(dbg_invyT[:], invyT[:])
                    nc.sync.dma_start(dbg_vT[:], vT_aug[:])
                    nc.sync.dma_start(dbg_attn[:], attn_sb[:])

    nc.compile()
    return nc


def _make_in_maps(inputs):
    import ml_dtypes
    bf16 = ml_dtypes.bfloat16

    x = np.ascontiguousarray(np.asarray(inputs["x"], np.float32)).reshape(B, CQ, N)
    y = np.ascontiguousarray(np.asarray(inputs["y"], np.float32)).reshape(B, CKV, N)
    q_w = np.asarray(inputs["q_w"], np.float32)
    kv_w = np.asarray(inputs["kv_w"], np.float32)
    qn = np.asarray(inputs["q_norm_w"], np.float32)
    kvn = np.asarray(inputs["kv_norm_w"], np.float32)
    q_b = np.asarray(inputs["q_b"], np.float32)
    kv_b = np.asarray(inputs["kv_b"], np.float32)
    proj_w = np.asarray(inputs["proj_w"], np.float32)
    proj_b = np.asarray(inputs["proj_b"], np.float32)

    qwT = np.ascontiguousarray((q_w * qn[None, :]).T).reshape(2, 128, 128)
    kvwT = np.ascontiguousarray((kv_w * kvn[None, :]).T).reshape(4, 128, 256)
    pwT = np.ascontiguousarray(proj_w.T)
    pb_eff = (proj_b + proj_w @ kv_b[128:]).astype(np.float32)
    shared = {
        "qwT": qwT.astype(bf16), "kvwT": kvwT.astype(bf16),
        "pwT": pwT.astype(bf16),
        "qb": q_b.reshape(128, 1),
        "pb": np.ascontiguousarray(pb_eff.reshape(2, 128).T),
        "ones": np.ones((128, 1), bf16),
    }
    in_maps = []
    for c in range(N_CORES):
        b, half = c // 2, c % 2
        xbv = np.ascontiguousarray(
            x[b][:, half * NH:(half + 1) * NH]).reshape(2, 128, NH)
        ybv = np.ascontiguousarray(y[b]).reshape(4, 128, N)
        in_maps.append({"xb": xbv.astype(bf16), "yb": ybv.astype(bf16),
                        **shared})
    return in_maps


class _Executor:
    """Compile once; run the SPMD kernel via PJRT/axon on 8 cores."""

    def __init__(self, reps=1):
        import jax
        from jax.sharding import Mesh, PartitionSpec
        from jax.experimental.shard_map import shard_map

        from concourse import bass2jax, mybir

        self.nc = _build_module(reps=reps)
        bass2jax.install_neuronx_cc_hook()

        partition_name = (self.nc.partition_id_tensor.name
                          if self.nc.partition_id_tensor else None)
        in_names, out_names, out_avals, zero_outs = [], [], [], []
        for alloc in self.nc.m.functions[0].allocations:
            if not isinstance(alloc, mybir.MemoryLocationSet):
                continue
            name = alloc.memorylocations[0].name
            if alloc.kind == "ExternalInput":
                if name != partition_name:
                    in_names.append(name)
            elif alloc.kind == "ExternalOutput":
                out_names.append(name)
                shape = tuple(alloc.tensor_shape)
                dtype = mybir.dt.np(alloc.dtype)
                out_avals.append(jax.core.ShapedArray(shape, dtype))
                zero_outs.append(np.zeros(shape, dtype))
        self.in_names, self.out_names = in_names, out_names
        self.out_avals, self.zero_outs = out_avals, zero_outs
        n_params, n_outs = len(in_names), len(out_names)
        all_names = list(in_names + out_names)
        if partition_name is not None:
            all_names.append(partition_name)
        all_names = tuple(all_names)

        def make_body(reps):
            def _body(*args):
                operands = list(args)
                if partition_name is not None:
                    operands.append(bass2jax.partition_id_tensor())
                return tuple(bass2jax._bass_exec_p.bind(
                    *operands,
                    out_avals=tuple(out_avals),
                    in_names=all_names,
                    out_names=tuple(out_names),
                    lowering_input_output_aliases=(),
                    sim_require_finite=False,
                    sim_require_nnan=False,
                    nc=self.nc,
                ))
            return _body

        devices = jax.devices()[:N_CORES]
        self.mesh = Mesh(np.asarray(devices), ("core",))
        in_specs = (PartitionSpec("core"),) * (n_params + n_outs)
        out_specs = (PartitionSpec("core"),) * n_outs
        self._jits = {}

        def get_jit(reps):
            if reps not in self._jits:
                self._jits[reps] = jax.jit(
                    shard_map(make_body(reps), mesh=self.mesh,
                              in_specs=in_specs, out_specs=out_specs,
                              check_rep=False),
                    keep_unused=True)
            return self._jits[reps]

        self._get_jit = get_jit

    def _concat_args(self, in_maps):
        cat = [np.concatenate([np.asarray(m[n]) for m in in_maps], axis=0)
               for n in self.in_names]
        cat += [np.concatenate([z] * len(in_maps), axis=0)
                for z in self.zero_outs]
        return cat

    def run(self, in_maps, reps=1):
        args = self._concat_args(in_maps)
        outs = self._get_jit(reps)(*args)
        n = len(in_maps)
        return [
            {name: np.asarray(outs[i]).reshape(n, *self.out_avals[i].shape)[c]
             for i, name in enumerate(self.out_names)}
            for c in range(n)
        ]


def _get_executor():
    global _EXEC
    if _EXEC is None:
        _EXEC = _Executor()
    return _EXEC


def kernel(**inputs):
    ex = _get_executor()
    res = ex.run(_make_in_maps(inputs))
    out = np.empty((B, CQ, N), np.float32)
    for c in range(N_CORES):
        b, half = c // 2, c % 2
        out[b][:, half * NH:(half + 1) * NH] = res[c]["o"].reshape(CQ, NH)
    return out.reshape(B, CQ, 48, 48)
